# revision 1
# baseline (speedup 1.0000x reference)
"""Trainium2 Bass kernel for nn_CrossAttentionSubLayer (sparse_attention).

Computes, per batch b:
    S = Q @ K^T / sqrt(d)            [Sq, Sk]
    S = where(kmask==0, ~0, S)       (reference fills with -1e-13 ~ 0)
    P = softmax(S, axis=-1)
    res = P @ V^T                    (V stored [d, Sk])
    out = res @ W + bias

Sharding: data-parallel over (batch, Sq/2) -> 8 shards, one per NeuronCore.
No collectives needed; the final Linear acts on the last dim.

Device layout trick: the PE contracts over the partition dim, so we compute
S^T = (K^T)^T.T.. i.e. ST[k, q] tiles with d-major operands.  All operands are
pre-transposed on the host during sharding (pure np layout work), so the
device kernel does zero transposes:
  - QT  [d, q]    (lhs-moving for ST)
  - KT  [d, k]    (stationary for ST)
  - VT  [k, d]    (stationary for PV; attn arrives naturally as [k, q])
  - W   [d, e]    (natural layout is already d-major)
Softmax over k is a partition-dim reduction, done with an all-ones matmul on
the PE (output rows all equal the sum).  Masking folds into the Exp
activation's per-partition scale: exp(scale[p] * s), scale in {0, 1/32}.
Masked positions get exp(0)=1, matching the reference's -1e-13 fill to fp32
precision.  Matmuls run in float32r (full PE speed, ~1.6e-4 rel err).
"""

import functools

import numpy as np

B, SQ, SK, D, E = 4, 2048, 2048, 1024, 1024
N_CORES = 8
QL = SQ // 2          # queries per core (shard)
KB = SK // 128        # 16 k-blocks
DC = D // 128         # 8 d-chunks
QT_TILES = QL // 512  # 2 q-tiles per core
SCALE = 1.0 / np.sqrt(np.float32(D))


@functools.lru_cache(maxsize=None)
def _build():
    import concourse.bass as bass
    import concourse.tile as tile
    from concourse import bacc, mybir

    F32 = mybir.dt.float32
    F32R = mybir.dt.float32r
    I32 = mybir.dt.int32
    EXP = mybir.ActivationFunctionType.Exp

    nc = bacc.Bacc("TRN2", target_bir_lowering=False, debug=False)

    kt_d = nc.dram_tensor("kt", [KB, 128, D], F32, kind="ExternalInput")
    qt_d = nc.dram_tensor("qt", [QT_TILES, DC, 128, 512], F32, kind="ExternalInput")
    vt_d = nc.dram_tensor("vt", [KB, 128, D], F32, kind="ExternalInput")
    w_d = nc.dram_tensor("w", [DC, 128, E], F32, kind="ExternalInput")
    mask_d = nc.dram_tensor("mask", [128, KB], I32, kind="ExternalInput")
    bias_d = nc.dram_tensor("bias", [E], F32, kind="ExternalInput")
    out_d = nc.dram_tensor("out", [QL, E], F32, kind="ExternalOutput")

    kt_ap = kt_d.ap().bitcast(F32R)
    qt_ap = qt_d.ap().bitcast(F32R)
    vt_ap = vt_d.ap().bitcast(F32R)
    w_ap = w_d.ap().bitcast(F32R)

    with tile.TileContext(nc) as tc:
        with (
            tc.tile_pool(name="const", bufs=1) as const,
            tc.tile_pool(name="big", bufs=1) as big,
            tc.tile_pool(name="psum", bufs=1, space="PSUM") as psum,
        ):
            # --- constants ---
            mask_i = const.tile([128, KB], I32)
            nc.sync.dma_start(mask_i[:], mask_d[:])
            mask_s = const.tile([128, KB], F32)
            # int32 -> fp32 cast, then scale by 1/sqrt(d)
            nc.vector.tensor_copy(mask_s[:], mask_i[:])
            nc.vector.tensor_scalar_mul(mask_s[:], mask_s[:], float(SCALE))

            ones_f = const.tile([128, 128], F32)
            nc.vector.memset(ones_f[:], 1.0)
            ones_r = const.tile([128, 128], F32R)
            nc.vector.tensor_copy(ones_r[:], ones_f[:])

            bias_b = const.tile([128, E], F32)
            bias_ap = bias_d.ap()
            nc.sync.dma_start(
                bias_b[:],
                bass.AP(tensor=bias_ap.tensor, offset=bias_ap.offset,
                        ap=[[0, 128]] + list(bias_ap.ap)),
            )

            # --- resident K^T blocks: [d-in-chunk, (c, k)] per k-block ---
            kt_t = []
            for kb in range(KB):
                t = big.tile([128, D], F32R, name=f"kt{kb}", tag="kt", bufs=KB)
                nc.sync.dma_start(t[:], kt_ap[kb])
                kt_t.append(t)

            for qt in range(QT_TILES):
                # --- per-q-tile Q^T chunks ---
                qtl = []
                for c in range(DC):
                    t = big.tile([128, 512], F32R, name=f"q{qt}_{c}", tag="qt", bufs=8)
                    nc.sync.dma_start(t[:], qt_ap[qt, c])
                    qtl.append(t)

                # --- phase A: ST = KT.T @ QT per k-block; P = exp(mask*scale*ST) ---
                p_t = []
                for kb in range(KB):
                    s_ps = psum.tile([128, 512], F32, name=f"s{qt}_{kb}",
                                     tag="acc", bufs=8)
                    for c in range(DC):
                        nc.tensor.matmul(
                            s_ps[:], kt_t[kb][:, c * 128:(c + 1) * 128], qtl[c][:],
                            start=(c == 0), stop=(c == DC - 1),
                        )
                    p = big.tile([128, 512], F32R, name=f"p{qt}_{kb}", tag="p",
                                 bufs=KB + 1)
                    nc.scalar.activation(p[:], s_ps[:], EXP,
                                         scale=mask_s[:, kb:kb + 1])
                    p_t.append(p)

                # --- rowsum over k (partition dim) via all-ones matmul ---
                rs_ps = psum.tile([128, 512], F32, name=f"rs{qt}", tag="acc", bufs=8)
                for kb in range(KB):
                    nc.tensor.matmul(rs_ps[:], ones_r[:], p_t[kb][:],
                                     start=(kb == 0), stop=(kb == KB - 1))
                recip = big.tile([128, 512], F32, name=f"recip{qt}", tag="recip",
                                 bufs=2)
                nc.vector.reciprocal(recip[:], rs_ps[:])

                # --- phase B: resT[c] = sum_k VT[k, c] * P[k, q] ---
                r_ps = [
                    psum.tile([128, 512], F32, name=f"r{qt}_{c}", tag="acc", bufs=8)
                    for c in range(DC)
                ]
                for kb in range(KB):
                    vt_t = big.tile([128, D], F32R, name=f"v{qt}_{kb}", tag="vt",
                                    bufs=3)
                    nc.sync.dma_start(vt_t[:], vt_ap[kb])
                    for c in range(DC):
                        nc.tensor.matmul(
                            r_ps[c][:], vt_t[:, c * 128:(c + 1) * 128], p_t[kb][:],
                            start=(kb == 0), stop=(kb == KB - 1),
                        )
                # normalize while copying PSUM -> SBUF
                r_t = []
                for c in range(DC):
                    t = big.tile([128, 512], F32R, name=f"rt{qt}_{c}", tag="rt",
                                 bufs=DC)
                    nc.vector.tensor_mul(t[:], r_ps[c][:], recip[:])
                    r_t.append(t)

                # --- phase C: O = resT.T @ W + bias ---
                for eh in range(E // 512):
                    wts = []
                    for c in range(DC):
                        t = big.tile([128, 512], F32R, name=f"w{qt}_{eh}_{c}",
                                     tag="w", bufs=9)
                        nc.sync.dma_start(t[:], w_ap[c, :, eh * 512:(eh + 1) * 512])
                        wts.append(t)
                    for qs in range(4):
                        o_ps = psum.tile([128, 512], F32, name=f"o{qt}_{eh}_{qs}",
                                         tag="acc", bufs=8)
                        for c in range(DC):
                            nc.tensor.matmul(
                                o_ps[:], r_t[c][:, qs * 128:(qs + 1) * 128], wts[c][:],
                                start=(c == 0), stop=(c == DC - 1),
                            )
                        o_t = big.tile([128, 512], F32, name=f"ot{qt}_{eh}_{qs}",
                                       tag="o", bufs=4)
                        nc.vector.tensor_add(o_t[:], o_ps[:],
                                             bias_b[:, eh * 512:(eh + 1) * 512])
                        row0 = qt * 512 + qs * 128
                        nc.sync.dma_start(
                            out_d[row0:row0 + 128, eh * 512:(eh + 1) * 512], o_t[:]
                        )

    nc.compile()
    return nc


def shard_inputs(Q, K, V, query_attention_mask, key_attention_mask, W, b):
    """Host-side shard + layout prep (pure np slicing/transpose)."""
    Q = np.ascontiguousarray(np.asarray(Q, dtype=np.float32))
    K = np.ascontiguousarray(np.asarray(K, dtype=np.float32))
    V = np.ascontiguousarray(np.asarray(V, dtype=np.float32))
    W = np.ascontiguousarray(np.asarray(W, dtype=np.float32))
    bias = np.ascontiguousarray(np.asarray(b, dtype=np.float32))
    kmask = np.asarray(key_attention_mask, dtype=np.int32)

    w_r = np.ascontiguousarray(W.reshape(DC, 128, E))
    in_maps = []
    per_batch = {}
    for core in range(N_CORES):
        bi, h = divmod(core, 2)
        if bi not in per_batch:
            kt = K[bi].T  # [D, SK]
            kt_r = np.ascontiguousarray(
                kt.reshape(DC, 128, KB, 128).transpose(2, 1, 0, 3)
            ).reshape(KB, 128, D)
            vt_r = np.ascontiguousarray(V[bi].T).reshape(KB, 128, D)
            mask_r = np.ascontiguousarray(kmask[bi].reshape(KB, 128).T)
            per_batch[bi] = (kt_r, vt_r, mask_r)
        kt_r, vt_r, mask_r = per_batch[bi]
        qt = Q[bi, h * QL:(h + 1) * QL].T  # [D, QL]
        qt_r = np.ascontiguousarray(
            qt.reshape(DC, 128, QT_TILES, 512).transpose(2, 0, 1, 3)
        )
        in_maps.append({
            "kt": kt_r, "qt": qt_r, "vt": vt_r, "w": w_r,
            "mask": mask_r, "bias": bias,
        })
    return in_maps


def unshard_output(results):
    out = np.empty((B, SQ, E), dtype=np.float32)
    for core in range(N_CORES):
        bi, h = divmod(core, 2)
        out[bi, h * QL:(h + 1) * QL] = results[core]["out"]
    return out


def kernel(Q, K, V, query_attention_mask, key_attention_mask, W, b):
    from concourse.bass_utils import run_bass_kernel_spmd

    nc = _build()
    in_maps = shard_inputs(Q, K, V, query_attention_mask, key_attention_mask, W, b)
    res = run_bass_kernel_spmd(nc, in_maps, list(range(N_CORES)))
    return unshard_output(res.results)


if __name__ == "__main__":
    rng = np.random.default_rng(0)
    inputs = {
        "Q": rng.standard_normal((B, SQ, D), dtype=np.float32),
        "K": rng.standard_normal((B, SK, D), dtype=np.float32),
        "V": rng.standard_normal((B, D, SK), dtype=np.float32),
        "query_attention_mask": np.ones((B, SQ), dtype=np.int32),
        "key_attention_mask": (rng.random((B, SK)) < 0.5).astype(np.int32),
        "W": rng.standard_normal((D, E), dtype=np.float32).astype(np.float32) / 32.0,
        "b": np.zeros(E, dtype=np.float32),
    }
    out = kernel(**inputs)
    print("out", out.shape, out.dtype, float(np.abs(out).max()))


# revision 4
# speedup vs baseline: 37.3200x; 37.3200x over previous
"""Trainium2 Bass kernel for nn_CrossAttentionSubLayer (sparse_attention).

Computes, per batch b:
    S = Q @ K^T / sqrt(d)            [Sq, Sk]
    S = where(kmask==0, ~0, S)       (reference fills with -1e-13 ~ 0)
    P = softmax(S, axis=-1)
    res = P @ V^T                    (V stored [d, Sk])
    out = res @ W + bias

Sharding: data-parallel over (batch, Sq/2) -> 8 shards, one per NeuronCore.
No collectives needed; the final Linear acts on the last dim.

Device layout trick: the PE contracts over the partition dim, so we compute
ST[k, q] = S^T tiles with d-major operands.  All operands are pre-transposed
on the host during sharding (pure np layout work), so the device kernel does
zero transposes:
  - QT  [d, q]    (moving operand for ST)
  - KT  [d, k]    (stationary for ST)
  - VT  [k, d]    (stationary for PV; attn arrives naturally as [k, q])
  - W   [d, e]    (natural layout is already d-major)
Softmax over k is a partition-dim reduction, done with an all-ones matmul on
the PE (output rows all equal the sum).  Masking folds into the Exp
activation's per-partition scale: exp(scale[p] * s), scale in {0, 1/32}.
Masked positions get exp(0)=1, matching the reference's -1e-13 fill to fp32
precision.  Matmuls run in float32r (full PE speed, ~1.6e-4 rel err).
"""

import functools

import numpy as np

B, SQ, SK, D, E = 4, 2048, 2048, 1024, 1024
N_CORES = 8
QL = SQ // 2          # queries per core (shard)
KB = SK // 128        # 16 k-blocks
DC = D // 128         # 8 d-chunks
QT_TILES = QL // 512  # 2 q-tiles per core
SCALE = 1.0 / np.sqrt(np.float32(D))


@functools.lru_cache(maxsize=None)
def _build(repeat: int = 1):
    import concourse.bass as bass
    import concourse.tile as tile
    from concourse import bacc, mybir

    F32 = mybir.dt.float32
    F32R = mybir.dt.float32r
    I32 = mybir.dt.int32
    EXP = mybir.ActivationFunctionType.Exp

    nc = bacc.Bacc("TRN2", target_bir_lowering=False, debug=False)

    kt_d = nc.dram_tensor("kt", [KB, 128, D], F32, kind="ExternalInput")
    qt_d = nc.dram_tensor("qt", [QT_TILES, DC, 128, 512], F32, kind="ExternalInput")
    vt_d = nc.dram_tensor("vt", [KB, 128, D], F32, kind="ExternalInput")
    w_d = nc.dram_tensor("w", [DC, 128, E], F32, kind="ExternalInput")
    mask_d = nc.dram_tensor("mask", [128, KB], I32, kind="ExternalInput")
    bias_d = nc.dram_tensor("bias", [E], F32, kind="ExternalInput")
    out_d = nc.dram_tensor("out", [QL, E], F32, kind="ExternalOutput")

    kt_ap = kt_d.ap().bitcast(F32R)
    qt_ap = qt_d.ap().bitcast(F32R)
    vt_ap = vt_d.ap().bitcast(F32R)
    w_ap = w_d.ap().bitcast(F32R)

    with tile.TileContext(nc) as tc:
        with (
            tc.tile_pool(name="const", bufs=1) as const,
            tc.tile_pool(name="big", bufs=1) as big,
            tc.tile_pool(name="psum", bufs=1, space="PSUM") as psum,
        ):
            # --- constants (outside the timing repeat loop) ---
            mask_i = const.tile([128, KB], I32)
            nc.sync.dma_start(mask_i[:], mask_d[:])
            mask_s = const.tile([128, KB], F32)
            # int32 -> fp32 cast, then scale by 1/sqrt(d)
            nc.vector.tensor_copy(mask_s[:], mask_i[:])
            nc.vector.tensor_scalar_mul(mask_s[:], mask_s[:], float(SCALE))

            ones_f = const.tile([128, 128], F32)
            nc.vector.memset(ones_f[:], 1.0)
            ones_r = const.tile([128, 128], F32R)
            nc.vector.tensor_copy(ones_r[:], ones_f[:])

            bias_b = const.tile([128, E], F32)
            bias_ap = bias_d.ap()
            nc.sync.dma_start(
                bias_b[:],
                bass.AP(tensor=bias_ap.tensor, offset=bias_ap.offset,
                        ap=[[0, 128]] + list(bias_ap.ap)),
            )

            def body():
                # --- K^T blocks: [d-in-chunk, (c, k)] per k-block ---
                kt_t = []
                for kb in range(KB):
                    t = big.tile([128, D], F32R, name=f"kt{kb}", tag="kt", bufs=KB)
                    nc.sync.dma_start(t[:], kt_ap[kb])
                    kt_t.append(t)

                for qt in range(QT_TILES):
                    # --- per-q-tile Q^T chunks ---
                    qtl = []
                    for c in range(DC):
                        t = big.tile([128, 512], F32R, name=f"q{qt}_{c}",
                                     tag="qt", bufs=8)
                        nc.sync.dma_start(t[:], qt_ap[qt, c])
                        qtl.append(t)

                    # --- phase A: ST = KT.T @ QT per k-block; P = exp(...) ---
                    p_t = []
                    for kb in range(KB):
                        s_ps = psum.tile([128, 512], F32, name=f"s{qt}_{kb}",
                                         tag="acc", bufs=8)
                        for c in range(DC):
                            nc.tensor.matmul(
                                s_ps[:], kt_t[kb][:, c * 128:(c + 1) * 128],
                                qtl[c][:],
                                start=(c == 0), stop=(c == DC - 1),
                            )
                        p = big.tile([128, 512], F32R, name=f"p{qt}_{kb}",
                                     tag="p", bufs=KB + 1)
                        nc.scalar.activation(p[:], s_ps[:], EXP,
                                             scale=mask_s[:, kb:kb + 1])
                        p_t.append(p)

                    # --- rowsum over k (partition dim) via all-ones matmul ---
                    rs_ps = psum.tile([128, 512], F32, name=f"rs{qt}",
                                      tag="acc", bufs=8)
                    for kb in range(KB):
                        nc.tensor.matmul(rs_ps[:], ones_r[:], p_t[kb][:],
                                         start=(kb == 0), stop=(kb == KB - 1))
                    recip = big.tile([128, 512], F32, name=f"recip{qt}",
                                     tag="recip", bufs=2)
                    nc.vector.reciprocal(recip[:], rs_ps[:])

                    # --- phase B: resT[c] = sum_k VT[k, c] * P[k, q] ---
                    r_ps = [
                        psum.tile([128, 512], F32, name=f"r{qt}_{c}",
                                  tag="acc", bufs=8)
                        for c in range(DC)
                    ]
                    for kb in range(KB):
                        vt_t = big.tile([128, D], F32R, name=f"v{qt}_{kb}",
                                        tag="vt", bufs=3)
                        nc.sync.dma_start(vt_t[:], vt_ap[kb])
                        for c in range(DC):
                            nc.tensor.matmul(
                                r_ps[c][:], vt_t[:, c * 128:(c + 1) * 128],
                                p_t[kb][:],
                                start=(kb == 0), stop=(kb == KB - 1),
                            )
                    # normalize while copying PSUM -> SBUF
                    r_t = []
                    for c in range(DC):
                        t = big.tile([128, 512], F32R, name=f"rt{qt}_{c}",
                                     tag="rt", bufs=DC)
                        nc.vector.tensor_mul(t[:], r_ps[c][:], recip[:])
                        r_t.append(t)

                    # --- phase C: O = resT.T @ W + bias ---
                    for eh in range(E // 512):
                        wts = []
                        for c in range(DC):
                            t = big.tile([128, 512], F32R,
                                         name=f"w{qt}_{eh}_{c}", tag="w", bufs=9)
                            nc.sync.dma_start(
                                t[:], w_ap[c, :, eh * 512:(eh + 1) * 512])
                            wts.append(t)
                        for qs in range(4):
                            o_ps = psum.tile([128, 512], F32,
                                             name=f"o{qt}_{eh}_{qs}",
                                             tag="acc", bufs=8)
                            for c in range(DC):
                                nc.tensor.matmul(
                                    o_ps[:],
                                    r_t[c][:, qs * 128:(qs + 1) * 128],
                                    wts[c][:],
                                    start=(c == 0), stop=(c == DC - 1),
                                )
                            o_t = big.tile([128, 512], F32,
                                           name=f"ot{qt}_{eh}_{qs}",
                                           tag="o", bufs=4)
                            nc.vector.tensor_add(
                                o_t[:], o_ps[:],
                                bias_b[:, eh * 512:(eh + 1) * 512])
                            row0 = qt * 512 + qs * 128
                            nc.sync.dma_start(
                                out_d[row0:row0 + 128,
                                      eh * 512:(eh + 1) * 512], o_t[:]
                            )

            if repeat == 1:
                body()
            else:
                with tc.For_i(0, repeat, 1):
                    body()

    nc.compile()
    return nc


def shard_inputs(Q, K, V, query_attention_mask, key_attention_mask, W, b):
    """Host-side shard + layout prep (pure np slicing/transpose)."""
    Q = np.ascontiguousarray(np.asarray(Q, dtype=np.float32))
    K = np.ascontiguousarray(np.asarray(K, dtype=np.float32))
    V = np.ascontiguousarray(np.asarray(V, dtype=np.float32))
    W = np.ascontiguousarray(np.asarray(W, dtype=np.float32))
    bias = np.ascontiguousarray(np.asarray(b, dtype=np.float32))
    kmask = np.asarray(key_attention_mask, dtype=np.int32)

    w_r = np.ascontiguousarray(W.reshape(DC, 128, E))
    in_maps = []
    per_batch = {}
    for core in range(N_CORES):
        bi, h = divmod(core, 2)
        if bi not in per_batch:
            kt = K[bi].T  # [D, SK]
            kt_r = np.ascontiguousarray(
                kt.reshape(DC, 128, KB, 128).transpose(2, 1, 0, 3)
            ).reshape(KB, 128, D)
            vt_r = np.ascontiguousarray(V[bi].T).reshape(KB, 128, D)
            mask_r = np.ascontiguousarray(kmask[bi].reshape(KB, 128).T)
            per_batch[bi] = (kt_r, vt_r, mask_r)
        kt_r, vt_r, mask_r = per_batch[bi]
        qt = Q[bi, h * QL:(h + 1) * QL].T  # [D, QL]
        qt_r = np.ascontiguousarray(
            qt.reshape(DC, 128, QT_TILES, 512).transpose(2, 0, 1, 3)
        )
        in_maps.append({
            "kt": kt_r, "qt": qt_r, "vt": vt_r, "w": w_r,
            "mask": mask_r, "bias": bias,
        })
    return in_maps


def unshard_output(results):
    out = np.empty((B, SQ, E), dtype=np.float32)
    for core in range(N_CORES):
        bi, h = divmod(core, 2)
        out[bi, h * QL:(h + 1) * QL] = results[core]["out"]
    return out


def kernel(Q, K, V, query_attention_mask, key_attention_mask, W, b):
    from concourse.bass_utils import run_bass_kernel_spmd

    nc = _build()
    in_maps = shard_inputs(Q, K, V, query_attention_mask, key_attention_mask, W, b)
    res = run_bass_kernel_spmd(nc, in_maps, list(range(N_CORES)))
    return unshard_output(res.results)


if __name__ == "__main__":
    rng = np.random.default_rng(0)
    inputs = {
        "Q": rng.standard_normal((B, SQ, D), dtype=np.float32),
        "K": rng.standard_normal((B, SK, D), dtype=np.float32),
        "V": rng.standard_normal((B, D, SK), dtype=np.float32),
        "query_attention_mask": np.ones((B, SQ), dtype=np.int32),
        "key_attention_mask": (rng.random((B, SK)) < 0.5).astype(np.int32),
        "W": rng.standard_normal((D, E), dtype=np.float32) / 32.0,
        "b": np.zeros(E, dtype=np.float32),
    }
    out = kernel(**inputs)
    print("out", out.shape, out.dtype, float(np.abs(out).max()))


# revision 9
# speedup vs baseline: 54.6438x; 1.4642x over previous
"""Trainium2 Bass kernel for nn_CrossAttentionSubLayer (sparse_attention).

Computes, per batch b:
    S = Q @ K^T / sqrt(d)            [Sq, Sk]
    S = where(kmask==0, ~0, S)       (reference fills with -1e-13 ~ 0)
    P = softmax(S, axis=-1)
    res = P @ V^T                    (V stored [d, Sk])
    out = res @ W + bias

Sharding: data-parallel over (batch, Sq/2) -> 8 shards, one per NeuronCore.
No collectives; the final Linear acts on the last dim.

Key structural points:
  * The PE contracts over the partition dim, so we compute ST[k, q] = S^T
    tiles with d-major operands.  All operands are pre-transposed on the host
    during sharding (pure np layout work) -> zero on-device transposes.
  * Mask sparsity (~50% of keys masked): a masked key contributes exp(~0)=1
    regardless of its score, i.e. a rank-1 term (1_q x sum_masked V[:,k]) to
    P@V^T and a constant N_masked to the softmax denominator.  So the host
    splits keys into compacted unmasked / masked groups (index gather only);
    the device computes QK/exp only over the unmasked ~9 blocks instead of
    16, adds the masked rank-1 term with a handful of K=1 matmuls, and adds
    N_masked (counted on device from a validity map) to the rowsum.
  * Compacted padding slots carry zero K/V columns and an exp bias of -1e30
    (exp -> 0), so they contribute nothing.
  * Softmax rowsum over k (partition dim) uses an all-ones stationary matmul
    (output rows all equal the sum).  Matmuls run in float32r (full PE speed,
    ~1.6e-4 rel err).  The max-subtraction is skipped: scores ~ N(0,1) and
    the reference softmax is mathematically shift-invariant.
"""

import functools

import numpy as np

B, SQ, SK, D, E = 4, 2048, 2048, 1024, 1024
N_CORES = 8
QL = SQ // 2          # queries per core (shard)
DC = D // 128         # 8 d-chunks
QT_TILES = QL // 512  # 2 q-tiles per core
SCALE = 1.0 / float(np.sqrt(np.float32(D)))
NEG_BIG = -1.0e30


@functools.lru_cache(maxsize=None)
def _build(kcb: int, mcb: int, repeat: int = 1):
    """kcb/mcb: number of 128-row blocks of compacted unmasked/masked keys."""
    import concourse.bass as bass
    import concourse.tile as tile
    from concourse import bacc, mybir

    F32 = mybir.dt.float32
    F32R = mybir.dt.float32r
    I32 = mybir.dt.int32
    EXP = mybir.ActivationFunctionType.Exp
    X = mybir.AxisListType.X
    ADD = mybir.AluOpType.add

    nc = bacc.Bacc("TRN2", target_bir_lowering=False, debug=False)

    kt_d = nc.dram_tensor("kt", [kcb, 128, D], F32, kind="ExternalInput")
    qt_d = nc.dram_tensor("qt", [QT_TILES, DC, 128, 512], F32, kind="ExternalInput")
    vtu_d = nc.dram_tensor("vtu", [kcb, 128, D], F32, kind="ExternalInput")
    vtm_d = nc.dram_tensor("vtm", [mcb, 128, D], F32, kind="ExternalInput")
    w_d = nc.dram_tensor("w", [DC, 128, E], F32, kind="ExternalInput")
    bexp_d = nc.dram_tensor("bexp", [128, kcb], F32, kind="ExternalInput")
    mvalid_d = nc.dram_tensor("mvalid", [128, mcb], I32, kind="ExternalInput")
    bias_d = nc.dram_tensor("bias", [E], F32, kind="ExternalInput")
    out_d = nc.dram_tensor("out", [QL, E], F32, kind="ExternalOutput")

    kt_ap = kt_d.ap().bitcast(F32R)
    qt_ap = qt_d.ap().bitcast(F32R)
    vtu_ap = vtu_d.ap().bitcast(F32R)
    vtm_ap = vtm_d.ap().bitcast(F32R)
    w_ap = w_d.ap().bitcast(F32R)

    with tile.TileContext(nc) as tc:
        with (
            tc.tile_pool(name="const", bufs=1) as const,
            tc.tile_pool(name="big", bufs=1) as big,
            tc.tile_pool(name="psum", bufs=1, space="PSUM") as psum,
        ):
            # --- constants ---
            bexp_t = const.tile([128, kcb], F32)
            nc.sync.dma_start(bexp_t[:], bexp_d[:])

            mpad = max(16, (mcb + 15) // 16 * 16)  # even N for the fp32r matmul
            mvalid_i = const.tile([128, mcb], I32)
            nc.sync.dma_start(mvalid_i[:], mvalid_d[:])
            mvalid_f32 = const.tile([128, mpad], F32)
            nc.vector.memset(mvalid_f32[:], 0.0)
            nc.vector.tensor_copy(mvalid_f32[:, :mcb], mvalid_i[:])
            mvalid_f = const.tile([128, mpad], F32R)
            nc.vector.tensor_copy(mvalid_f[:], mvalid_f32[:])

            ones_f = const.tile([128, 128], F32)
            nc.vector.memset(ones_f[:], 1.0)
            ones_r = const.tile([128, 128], F32R)
            nc.vector.tensor_copy(ones_r[:], ones_f[:])
            ones_col = const.tile([128, 1], F32R)
            nc.vector.tensor_copy(ones_col[:], ones_f[:, :1])
            ones_row_f = const.tile([1, 512], F32)
            nc.vector.memset(ones_row_f[:], 1.0)
            ones_row = const.tile([1, 512], F32R)
            nc.vector.tensor_copy(ones_row[:], ones_row_f[:])

            bias_b = const.tile([128, E], F32)
            bias_ap = bias_d.ap()
            nc.sync.dma_start(
                bias_b[:],
                bass.AP(tensor=bias_ap.tensor, offset=bias_ap.offset,
                        ap=[[0, 128]] + list(bias_ap.ap)),
            )

            def body():
                # --- N_masked (scalar, as per-partition column) ---
                nm_ps = psum.tile([128, mpad], F32, name="nm_ps", tag="acc", bufs=8)
                nc.tensor.matmul(nm_ps[:], ones_r[:], mvalid_f[:],
                                 start=True, stop=True)
                nm_col = big.tile([128, 1], F32, name="nm_col", tag="nm", bufs=1)
                nc.vector.tensor_reduce(nm_col[:], nm_ps[:], axis=X, op=ADD)

                # --- maskV row: sum over masked keys of V^T, as [1, D] ---
                mv_row = big.tile([1, D], F32R, name="mv_row", tag="mv", bufs=1)
                mv_ps = [
                    psum.tile([1, 512], F32, name=f"mv_ps{dh}", tag="acc", bufs=8)
                    for dh in range(D // 512)
                ]
                for kb in range(mcb):
                    vtm_t = big.tile([128, D], F32R, name=f"vm{kb}",
                                     tag="vt", bufs=3)
                    nc.sync.dma_start(vtm_t[:], vtm_ap[kb])
                    for dh in range(D // 512):
                        nc.tensor.matmul(mv_ps[dh][:], ones_col[:],
                                         vtm_t[:, dh * 512:(dh + 1) * 512],
                                         start=(kb == 0), stop=(kb == mcb - 1))
                for dh in range(D // 512):
                    nc.vector.tensor_copy(mv_row[:, dh * 512:(dh + 1) * 512],
                                          mv_ps[dh][:])

                # --- K^T blocks (compacted unmasked) ---
                kt_t = []
                for kb in range(kcb):
                    t = big.tile([128, D], F32R, name=f"kt{kb}", tag="kt", bufs=kcb)
                    nc.sync.dma_start(t[:], kt_ap[kb])
                    kt_t.append(t)

                for qt in range(QT_TILES):
                    qtl = []
                    for c in range(DC):
                        t = big.tile([128, 512], F32R, name=f"q{qt}_{c}",
                                     tag="qt", bufs=8)
                        nc.sync.dma_start(t[:], qt_ap[qt, c])
                        qtl.append(t)

                    # --- phase A: ST = KT.T @ QT; P = exp(s/32 + bexp) ---
                    p_t = []
                    for kb in range(kcb):
                        s_ps = psum.tile([128, 512], F32, name=f"s{qt}_{kb}",
                                         tag="acc", bufs=8)
                        for c in range(DC):
                            nc.tensor.matmul(
                                s_ps[:], kt_t[kb][:, c * 128:(c + 1) * 128],
                                qtl[c][:],
                                start=(c == 0), stop=(c == DC - 1),
                            )
                        p = big.tile([128, 512], F32R, name=f"p{qt}_{kb}",
                                     tag="p", bufs=kcb + 1)
                        nc.scalar.activation(p[:], s_ps[:], EXP,
                                             bias=bexp_t[:, kb:kb + 1],
                                             scale=float(SCALE))
                        p_t.append(p)

                    # --- rowsum over compacted k + N_masked; reciprocal ---
                    rs_ps = psum.tile([128, 512], F32, name=f"rs{qt}",
                                      tag="acc", bufs=8)
                    for kb in range(kcb):
                        nc.tensor.matmul(rs_ps[:], ones_r[:], p_t[kb][:],
                                         start=(kb == 0), stop=(kb == kcb - 1))
                    rs_sb = big.tile([128, 512], F32, name=f"rssb{qt}",
                                     tag="rssb", bufs=2)
                    nc.vector.tensor_scalar_add(rs_sb[:], rs_ps[:], nm_col[:])
                    recip = big.tile([128, 512], F32, name=f"recip{qt}",
                                     tag="recip", bufs=2)
                    nc.vector.reciprocal(recip[:], rs_sb[:])

                    # --- phase B: resT[c] = sum_k VTu[k, c] P[k, q] + maskV ---
                    r_ps = [
                        psum.tile([128, 512], F32, name=f"r{qt}_{c}",
                                  tag="acc", bufs=8)
                        for c in range(DC)
                    ]
                    for kb in range(kcb):
                        vtu_t = big.tile([128, D], F32R, name=f"vu{qt}_{kb}",
                                         tag="vt", bufs=3)
                        nc.sync.dma_start(vtu_t[:], vtu_ap[kb])
                        for c in range(DC):
                            nc.tensor.matmul(
                                r_ps[c][:], vtu_t[:, c * 128:(c + 1) * 128],
                                p_t[kb][:],
                                start=(kb == 0), stop=False,
                            )
                    # rank-1 masked contribution: maskV[c] (x) ones_row
                    for c in range(DC):
                        nc.tensor.matmul(
                            r_ps[c][:], mv_row[:, c * 128:(c + 1) * 128],
                            ones_row[:],
                            start=False, stop=True,
                        )
                    # normalize while copying PSUM -> SBUF
                    r_t = []
                    for c in range(DC):
                        t = big.tile([128, 512], F32R, name=f"rt{qt}_{c}",
                                     tag="rt", bufs=DC)
                        nc.vector.tensor_mul(t[:], r_ps[c][:], recip[:])
                        r_t.append(t)

                    # --- phase C: O = resT.T @ W + bias ---
                    for eh in range(E // 512):
                        wts = []
                        for c in range(DC):
                            t = big.tile([128, 512], F32R,
                                         name=f"w{qt}_{eh}_{c}", tag="w", bufs=9)
                            nc.sync.dma_start(
                                t[:], w_ap[c, :, eh * 512:(eh + 1) * 512])
                            wts.append(t)
                        for qs in range(4):
                            o_ps = psum.tile([128, 512], F32,
                                             name=f"o{qt}_{eh}_{qs}",
                                             tag="acc", bufs=8)
                            for c in range(DC):
                                nc.tensor.matmul(
                                    o_ps[:],
                                    r_t[c][:, qs * 128:(qs + 1) * 128],
                                    wts[c][:],
                                    start=(c == 0), stop=(c == DC - 1),
                                )
                            o_t = big.tile([128, 512], F32,
                                           name=f"ot{qt}_{eh}_{qs}",
                                           tag="o", bufs=4)
                            nc.vector.tensor_add(
                                o_t[:], o_ps[:],
                                bias_b[:, eh * 512:(eh + 1) * 512])
                            row0 = qt * 512 + qs * 128
                            nc.sync.dma_start(
                                out_d[row0:row0 + 128,
                                      eh * 512:(eh + 1) * 512], o_t[:]
                            )

            if repeat == 1:
                body()
            else:
                with tc.For_i(0, repeat, 1):
                    body()

    nc.compile()
    return nc


def _plan_blocks(kmask):
    """Per-batch unmasked/masked index lists + global block counts."""
    idx_u, idx_m = [], []
    for bi in range(B):
        m = kmask[bi] != 0
        idx_u.append(np.nonzero(m)[0])
        idx_m.append(np.nonzero(~m)[0])
    kcb = max(1, max((len(i) + 127) // 128 for i in idx_u))
    mcb = max(1, max((len(i) + 127) // 128 for i in idx_m))
    return idx_u, idx_m, kcb, mcb


def shard_inputs(Q, K, V, query_attention_mask, key_attention_mask, W, b):
    """Host-side shard + layout prep (np slicing / transpose / index gather)."""
    Q = np.ascontiguousarray(np.asarray(Q, dtype=np.float32))
    K = np.ascontiguousarray(np.asarray(K, dtype=np.float32))
    V = np.ascontiguousarray(np.asarray(V, dtype=np.float32))
    W = np.ascontiguousarray(np.asarray(W, dtype=np.float32))
    bias = np.ascontiguousarray(np.asarray(b, dtype=np.float32))
    kmask = np.asarray(key_attention_mask, dtype=np.int32)

    idx_u, idx_m, kcb, mcb = _plan_blocks(kmask)
    kc, mc = kcb * 128, mcb * 128

    w_r = np.ascontiguousarray(W.reshape(DC, 128, E))
    in_maps = []
    per_batch = {}
    for core in range(N_CORES):
        bi, h = divmod(core, 2)
        if bi not in per_batch:
            iu, im = idx_u[bi], idx_m[bi]
            nu, nm = len(iu), len(im)
            kt_full = K[bi].T  # [D, SK]
            ktc = np.zeros((D, kc), dtype=np.float32)
            ktc[:, :nu] = kt_full[:, iu]
            kt_r = np.ascontiguousarray(
                ktc.reshape(DC, 128, kcb, 128).transpose(2, 1, 0, 3)
            ).reshape(kcb, 128, D)

            vt_full = V[bi].T  # [SK, D]
            vtu = np.zeros((kc, D), dtype=np.float32)
            vtu[:nu] = vt_full[iu]
            vtu_r = vtu.reshape(kcb, 128, D)
            vtm = np.zeros((mc, D), dtype=np.float32)
            vtm[:nm] = vt_full[im]
            vtm_r = vtm.reshape(mcb, 128, D)

            bexp = np.full(kc, NEG_BIG, dtype=np.float32)
            bexp[:nu] = 0.0
            bexp_r = np.ascontiguousarray(bexp.reshape(kcb, 128).T)

            mvalid = np.zeros(mc, dtype=np.int32)
            mvalid[:nm] = 1
            mvalid_r = np.ascontiguousarray(mvalid.reshape(mcb, 128).T)

            per_batch[bi] = (kt_r, vtu_r, vtm_r, bexp_r, mvalid_r)
        kt_r, vtu_r, vtm_r, bexp_r, mvalid_r = per_batch[bi]
        qt = Q[bi, h * QL:(h + 1) * QL].T  # [D, QL]
        qt_r = np.ascontiguousarray(
            qt.reshape(DC, 128, QT_TILES, 512).transpose(2, 0, 1, 3)
        )
        in_maps.append({
            "kt": kt_r, "qt": qt_r, "vtu": vtu_r, "vtm": vtm_r, "w": w_r,
            "bexp": bexp_r, "mvalid": mvalid_r, "bias": bias,
        })
    return in_maps, kcb, mcb


def unshard_output(results):
    out = np.empty((B, SQ, E), dtype=np.float32)
    for core in range(N_CORES):
        bi, h = divmod(core, 2)
        out[bi, h * QL:(h + 1) * QL] = results[core]["out"]
    return out


def kernel(Q, K, V, query_attention_mask, key_attention_mask, W, b):
    from concourse.bass_utils import run_bass_kernel_spmd

    in_maps, kcb, mcb = shard_inputs(Q, K, V, query_attention_mask,
                                     key_attention_mask, W, b)
    nc = _build(kcb, mcb)
    res = run_bass_kernel_spmd(nc, in_maps, list(range(N_CORES)))
    return unshard_output(res.results)


if __name__ == "__main__":
    rng = np.random.default_rng(0)
    inputs = {
        "Q": rng.standard_normal((B, SQ, D), dtype=np.float32),
        "K": rng.standard_normal((B, SK, D), dtype=np.float32),
        "V": rng.standard_normal((B, D, SK), dtype=np.float32),
        "query_attention_mask": np.ones((B, SQ), dtype=np.int32),
        "key_attention_mask": (rng.random((B, SK)) < 0.5).astype(np.int32),
        "W": rng.standard_normal((D, E), dtype=np.float32) / 32.0,
        "b": np.zeros(E, dtype=np.float32),
    }
    out = kernel(**inputs)
    print("out", out.shape, out.dtype, float(np.abs(out).max()))
